# revision 1
# baseline (speedup 1.0000x reference)
"""Trainium2 Bass kernel for nn_CrossAttention (B=8, N=M=2048, C=512, H=4).

Sharding: data-parallel over batch — one batch element per NeuronCore (8 cores).
Per-core dataflow (layouts chosen so every matmul contracts over the partition
dim; fp16 operands everywhere -> 1-cycle/row PE + fast weight load; fp32 PSUM
accumulation throughout):

  1. F1^T, F2^T via PE transposes (fp32 exact), evacuated to fp16.
  2. q^T  = (F1 @ W + b)^T   : lhsT=W-chunk,  rhs=F1^T  -> [d-major fp16]
     kv^T = (F2 @ W + b)^T   : lhsT=W-chunk,  rhs=F2^T  -> [d-major fp16]
     kv   =  F2 @ W + b      : lhsT=F2^T-blk, rhs=W     -> [m-major fp16]
     (q^T/kv^T bias fused into the PSUM->SBUF evac as tensor_scalar_add
      with a per-partition bias column; kv/out bias via K=1 rank-1 matmuls)
  3. per (head, n-stripe of 512):
       scores^T[m,n] = kv_h^T.T @ q_h^T   (PSUM, 2 banks per pair of m-blocks)
       E^T = exp(SCALE * scores^T)        (ACT, PSUM->SBUF, fp16; no max-sub:
                                           |SCALE*s| <= ~2, exp safe in fp32)
       unnorm^T[d,n] = sum_m kv_h-blocks.T @ E^T   (PE, fp16 in, f32 acc)
       denom[1,n]    = sum_m ones[m,1].T @ E^T     (PE ones-matmuls)
       recip = 1/denom (DVE); partition-broadcast on GPSIMD (off PE path!)
       x^T[d,n] = unnorm^T * bcast(recip)  (DVE mul, writes fp16)
  4. out[n,c] = x^T-blocks.T @ W_proj + b_proj (K=1 matmul bias), DMA out.
"""
import sys

for _p in ("/opt/trn_rl_repo", "/root/.axon_site/_ro/trn_rl_repo"):
    if _p not in sys.path:
        sys.path.insert(0, _p)

import numpy as np
import concourse.bass as bass
import concourse.bacc as bacc
import concourse.tile as tile
from concourse import mybir
from concourse.bass_utils import run_bass_kernel_spmd

F32 = mybir.dt.float32
F16 = mybir.dt.float16
EXP = mybir.ActivationFunctionType.Exp

B, N, M, C = 8, 2048, 2048, 512
H, D = 4, 128
SCALE = 1.0 / np.sqrt(C)
P = 128
NB = N // P        # 16 n-blocks
MB = M // P        # 16 m-blocks
KC = C // P        # 4 contraction chunks (also = heads since D=128)
NS = 4             # n-stripes of 512
SW = N // NS       # stripe width 512


def build_nc():
    nc = bacc.Bacc(None, target_bir_lowering=False)
    dF1 = nc.dram_tensor("F1", [N, C], F32, kind="ExternalInput")
    dF2 = nc.dram_tensor("F2", [M, C], F32, kind="ExternalInput")
    dW = nc.dram_tensor("Wqkv", [C, C], F32, kind="ExternalInput")
    dBq = nc.dram_tensor("bqkv", [1, C], F32, kind="ExternalInput")
    dWp = nc.dram_tensor("Wproj", [C, C], F32, kind="ExternalInput")
    dBp = nc.dram_tensor("bproj", [1, C], F32, kind="ExternalInput")
    dOut = nc.dram_tensor("OUT", [N, C], F32, kind="ExternalOutput")

    d_ident = nc.inline_tensor(np.eye(P, dtype=np.float32), name="identity")
    d_ident16 = nc.inline_tensor(np.eye(P, dtype=np.float16), name="identity16")
    d_ones_row = nc.inline_tensor(np.ones((1, C), np.float16), name="ones_row")
    d_ones_col = nc.inline_tensor(np.ones((P, 1), np.float16), name="ones_col")

    with tile.TileContext(nc) as tc:
        with (
            tc.tile_pool(name="const", bufs=1) as const,
            tc.tile_pool(name="persist", bufs=1) as persist,
        ):
            # ---- constants / weights (fp16 via casting gpsimd DMA) ----
            ident = const.tile([P, P], F32)
            nc.sync.dma_start(ident, d_ident[:])
            ident16 = const.tile([P, P], F16)
            nc.sync.dma_start(ident16, d_ident16[:])
            ones_row = const.tile([1, C], F16)
            nc.sync.dma_start(ones_row, d_ones_row[:])
            ones_col = const.tile([P, 1], F16)
            nc.sync.dma_start(ones_col, d_ones_col[:])
            bq_row = const.tile([1, C], F16)
            nc.gpsimd.dma_start(bq_row, dBq[:])
            bq_col = const.tile([P, KC], F32)
            nc.sync.dma_start(
                bq_col, dBq[0, :].rearrange("(a b) -> b a", b=P)
            )
            bp_row = const.tile([1, C], F16)
            nc.gpsimd.dma_start(bp_row, dBp[:])
            W = []   # W[kc] = W_qkv[kc*128:(kc+1)*128, :]  [c-chunk, c_out]
            Wp = []
            for kc in range(KC):
                w = const.tile([P, C], F16, name=f"w{kc}")
                nc.gpsimd.dma_start(w, dW[kc * P:(kc + 1) * P, :])
                W.append(w)
                wp = const.tile([P, C], F16, name=f"wp{kc}")
                nc.gpsimd.dma_start(wp, dWp[kc * P:(kc + 1) * P, :])
                Wp.append(wp)

            # ---- persistent activations ----
            qT = [persist.tile([P, N], F16, name=f"qT{i}") for i in range(KC)]
            kvT = [persist.tile([P, M], F16, name=f"kvT{i}") for i in range(KC)]
            kvn = [persist.tile([P, C], F16, name=f"kvn{i}") for i in range(MB)]

            # ---- phase 1: F1^T, F2^T (PE transposes fp32, evac to fp16) ----
            with (
                tc.tile_pool(name="ftile", bufs=6) as fpool,
                tc.tile_pool(name="ftp", bufs=1) as ftp,
            ):
                FT = {}
                with tc.tile_pool(name="trps", bufs=8, space="PSUM") as trps:
                    for tag, dsrc in (("f1", dF1), ("f2", dF2)):
                        FT[tag] = [
                            ftp.tile([P, N], F16, name=f"{tag}T{i}")
                            for i in range(KC)
                        ]
                        for g in range(NS):
                            tp = [
                                trps.tile([P, SW], F32, tag="trp",
                                          name=f"trp_{tag}_{g}_{k}")
                                for k in range(KC)
                            ]
                            for i in range(4):
                                nb = 4 * g + i
                                fin = fpool.tile([P, C], F32, tag="fin")
                                nc.sync.dma_start(
                                    fin, dsrc[nb * P:(nb + 1) * P, :]
                                )
                                for kc in range(KC):
                                    nc.tensor.transpose(
                                        tp[kc][:, i * P:(i + 1) * P],
                                        fin[:, kc * P:(kc + 1) * P],
                                        ident,
                                    )
                            for kc in range(KC):
                                nc.vector.tensor_copy(
                                    FT[tag][kc][:, g * SW:(g + 1) * SW],
                                    tp[kc],
                                )

                # ---- phase 2: projections ----
                with tc.tile_pool(name="pjps", bufs=8, space="PSUM") as pjps:
                    # q^T and kv^T (d-major): lhsT = W[:, co-chunk], rhs = F^T
                    # head-0 chunks and kvn first so attention can start early
                    def emit_qkvT(dst, src, co):
                            for g in range(NS):
                                pj = pjps.tile([P, SW], F32, tag="pj", bufs=6)
                                for kc in range(KC):
                                    nc.tensor.matmul(
                                        pj,
                                        W[kc][:, co * P:(co + 1) * P],
                                        src[kc][:, g * SW:(g + 1) * SW],
                                        start=(kc == 0),
                                        stop=(kc == KC - 1),
                                    )
                                nc.vector.tensor_scalar_add(
                                    dst[co][:, g * SW:(g + 1) * SW],
                                    pj,
                                    bq_col[:, co:co + 1],
                                )
                    for co in range(KC):
                        emit_qkvT(qT, FT["f1"], co)
                    for co in range(KC):
                        emit_qkvT(kvT, FT["f2"], co)
                    # kv natural (m-major): transpose kv^T blocks (bias
                    # already folded into kv^T)
                    for mb in range(MB):
                        pjt = pjps.tile([P, C], F16, tag="pjt", bufs=2)
                        for hh in range(H):
                            nc.tensor.transpose(
                                pjt[:, hh * P:(hh + 1) * P],
                                kvT[hh][:, mb * P:(mb + 1) * P],
                                ident16,
                            )
                        nc.vector.tensor_copy(kvn[mb], pjt)

            # ---- phases 3+4 ----
            with tc.tile_pool(name="xtp", bufs=1) as xtp:
              xT = [xtp.tile([P, N], F16, name=f"xT{i}") for i in range(KC)]
              # ---- phase 3: attention per (head, n-stripe) ----
              with (
                tc.tile_pool(name="et", bufs=2) as epool,
                tc.tile_pool(name="scps", bufs=2, space="PSUM") as scps,
                tc.tile_pool(name="pvps", bufs=2, space="PSUM") as pvps,
                tc.tile_pool(name="dnps", bufs=2, space="PSUM") as dnps,
                tc.tile_pool(name="sm", bufs=2) as sm,
              ):
                for h in range(H):
                    for s in range(NS):
                        E = epool.tile([P, MB, SW], F16, tag="E")
                        pv = pvps.tile([P, SW], F32, tag="pv")
                        dn = dnps.tile([1, SW], F32, tag="dn")

                        def pv_dn_pair(jj):
                            for mb in (2 * jj, 2 * jj + 1):
                                nc.tensor.matmul(
                                    pv,
                                    kvn[mb][:, h * P:(h + 1) * P],
                                    E[:, mb, :],
                                    start=(mb == 0),
                                    stop=(mb == MB - 1),
                                )
                            for mb in (2 * jj, 2 * jj + 1):
                                nc.tensor.matmul(
                                    dn,
                                    ones_col,
                                    E[:, mb, :],
                                    start=(mb == 0),
                                    stop=(mb == MB - 1),
                                )

                        for j in range(MB // 2):
                            sc = scps.tile([P, 2, SW], F32, tag="sc")
                            for i in range(2):
                                mb = 2 * j + i
                                nc.tensor.matmul(
                                    sc[:, i, :],
                                    kvT[h][:, mb * P:(mb + 1) * P],
                                    qT[h][:, s * SW:(s + 1) * SW],
                                    start=True,
                                    stop=True,
                                )
                            # exp over both banks in one ACT instruction
                            nc.scalar.activation(
                                E[:, 2 * j:2 * j + 2, :].rearrange(
                                    "p a b -> p (a b)"
                                ),
                                sc.rearrange("p a b -> p (a b)"),
                                EXP,
                                scale=float(SCALE),
                            )
                            if j > 0:
                                pv_dn_pair(j - 1)
                        pv_dn_pair(MB // 2 - 1)
                        dns = sm.tile([1, SW], F32, tag="dns")
                        nc.vector.tensor_copy(dns, dn)
                        dnb = sm.tile([P, SW], F32, tag="dnb")
                        nc.gpsimd.partition_broadcast(dnb, dns)
                        recip = sm.tile([P, SW], F32, tag="recip")
                        nc.vector.reciprocal(recip, dnb)
                        with nc.allow_low_precision(
                            reason="x values O(0.1); fp16 keeps 5e-4 rel"
                        ):
                            nc.vector.tensor_mul(
                                xT[h][:, s * SW:(s + 1) * SW], pv, recip
                            )

              # ---- phase 4: output projection ----
              with (
                tc.tile_pool(name="prps", bufs=4, space="PSUM") as prps,
                tc.tile_pool(name="osb", bufs=3) as osb,
              ):
                for nb in range(NB):
                    pr = prps.tile([P, C], F32, tag="pr")
                    for kc in range(KC):
                        nc.tensor.matmul(
                            pr,
                            xT[kc][:, nb * P:(nb + 1) * P],
                            Wp[kc],
                            start=(kc == 0),
                            stop=False,
                        )
                    nc.tensor.matmul(
                        pr, ones_row[:, 0:P], bp_row, start=False, stop=True
                    )
                    ot = osb.tile([P, C], F32, tag="ot")
                    nc.scalar.copy(ot, pr)
                    nc.sync.dma_start(dOut[nb * P:(nb + 1) * P, :], ot)

    nc.compile()
    return nc


_NC = None


def _get_nc():
    global _NC
    if _NC is None:
        _NC = build_nc()
    return _NC


def kernel(F1, F2, W_qkv, b_qkv, W_proj, b_proj, _trace=False):
    F1 = np.ascontiguousarray(np.asarray(F1, dtype=np.float32))
    F2 = np.ascontiguousarray(np.asarray(F2, dtype=np.float32))
    W = np.ascontiguousarray(np.asarray(W_qkv, dtype=np.float32))
    bq = np.ascontiguousarray(np.asarray(b_qkv, dtype=np.float32)).reshape(1, C)
    Wpj = np.ascontiguousarray(np.asarray(W_proj, dtype=np.float32))
    bp = np.ascontiguousarray(np.asarray(b_proj, dtype=np.float32)).reshape(1, C)

    nc = _get_nc()
    in_maps = [
        {"F1": F1[b], "F2": F2[b], "Wqkv": W, "bqkv": bq, "Wproj": Wpj, "bproj": bp}
        for b in range(B)
    ]
    res = run_bass_kernel_spmd(
        nc, in_maps, core_ids=list(range(B)), trace=_trace
    )
    out = np.stack([res.results[b]["OUT"] for b in range(B)], axis=0)
    if _trace:
        return out, res
    return out

